# revision 58
# baseline (speedup 1.0000x reference)
"""Trainium2 Bass kernel for nn_ESBN_77352361001553 (scatter_memory).

Math: the conv encoder is dead code and the LSTM input is constant zeros, so
every batch row follows the identical 16-step, 512-dim LSTM trajectory from
zero state. Output (16, 1024, 4) = broadcast of out_t = Wo @ h_t + bo across
the batch; each of the 8 cores produces the same (16, 512)-f32 row block and
the host reshapes/concats to (16, 1024, 4).

Hand-scheduled raw-bass kernel (no Tile framework), ~58us vs the 73us Tile
baseline. Per-step period ~2.68us = f+g+i matmul phase (48 pairs @27ns,
the N=1 matmul issue floor) + the serial tail (PSUM drain+sem 0.27,
tanh_gi 0.30, v 0.19, D' 0.20, tanh(D/2) 0.27, h-update 0.19, sems ~0.15);
the o-group matmuls, tanh_o, and the NEXT step's three bias matmuls
(pre-issued, gated on the current step's ACT reads of their banks) all
overlap the tail, so each step opens directly with its f-matmuls.
Design notes:
 - tanh-only gates: host pre-scales f/i/o rows of Whh by 0.5 so
   tanh(pre) = 2*sigmoid(gate)-1, and tracks D = 2*cx, h2 = 2*h (the factor
   is folded into Whh's columns and Wo). Per step ACT runs 4 instrs
   (tanh_f, tanh_{g,i}, tanh_o, tanh(D/2)); DVE runs 4 fused
   scalar_tensor_tensor ops:
     u = (1+tf)*D ; v = (1+ti)*tg ; D' = 0.5*u + v ; h2 = (1+to)*th
 - weights are float8 e3m4 scaled by 16 (host-quantized; rhs stays fp16 —
   mixed-dtype matmul works); the 1/16 rides the tanh ACT scale for free.
   This halves the weight-DMA ramp; rel err 6.1e-3 (tolerance 2e-2).
 - gate biases enter PSUM via K=4/K=8 (bias^T, I) matmuls opening each
   bank's accumulation group (no bias-add on the critical path).
 - three PSUM gate banks (f | g+i | o): ACT reads a bank only after its
   matmuls stop, while PE writes a different bank (PSUM R/W collision is
   fatal); tanh(D/2) lands in its own PSUM bank (ScE->PSUM is faster).
 - head computed in two [8,4] PSUM chunks (steps 0-7 during step 9, steps
   8-15 at the end) so most of the output DMA overlaps compute.
 - hand-placed semaphores, one per producer stream + one per input DMA
   (completions of different DMAs interleave across the 16 SDMA engines, so
   a shared counter cannot order them). Every instruction carries <=1 wait.
 - the DVE does NOT interlock same-engine read-after-write: any DVE op
   reading the previous DVE op's output needs a wait on the DVE's own
   semaphore (D' above). Engine-FIFO completion order is still in-order.
 - standalone engine.wait_ge() fuses into the next instruction; on the PE
   that is the LDWEIGHTS half of a matmul, which is exactly where DMA
   gating must sit (the PE reorder window pulls LDWEIGHTS ahead of waiting
   matmuls).
Remaining fixed costs outside our control: ~1.4us of framework preamble
inside the measured window, and ~7.3us of neuronxcc-emitted full
semaphore-file resets + barriers at kernel end (the Tile baseline pays the
same).
"""

import os

import numpy as np

T = 16
HID = 512
N_CORES = 8
BSH = 128  # batch shard per core

WDT = os.environ.get("KERNEL_WDT", "f8")  # "f16" or "f8" (float8e3 = e3m4)
WSCALE = 16.0 if WDT == "f8" else 1.0
N_WARM_MM = int(os.environ.get("KERNEL_WARM", "56"))

_BUILT = {}
last_results = None  # BassKernelResults of the most recent run (for tooling)


def _ensure_ntff_hook():
    """Register the axon NTFF profiling hook if the container lacks
    antenv.axon_hooks (slim boot)."""
    import contextlib
    import ctypes
    import sys
    import types

    try:
        from antenv.axon_hooks import get_axon_ntff_profile_hook  # noqa: F401

        return
    except ImportError:
        pass

    so_path = "/opt/axon/libaxon_pjrt.so"
    hook = None
    if os.path.exists(so_path):
        lib = ctypes.CDLL(so_path)
        if hasattr(lib, "axon_start_nrt_profile"):
            lib.axon_start_nrt_profile.argtypes = [
                ctypes.POINTER(ctypes.c_int64),
                ctypes.c_size_t,
            ]
            lib.axon_start_nrt_profile.restype = ctypes.c_int64
            lib.axon_stop_nrt_profile.argtypes = [ctypes.c_char_p]
            lib.axon_stop_nrt_profile.restype = ctypes.c_int64

            @contextlib.contextmanager
            def _hook(output_dir, device_ids):
                import jax

                jax.devices()
                if device_ids:
                    ids = (ctypes.c_int64 * len(device_ids))(*device_ids)
                    rc = lib.axon_start_nrt_profile(ids, len(device_ids))
                else:
                    rc = lib.axon_start_nrt_profile(None, 0)
                if rc != 0:
                    raise RuntimeError(f"axon_start_nrt_profile rc={rc}")
                try:
                    yield
                finally:
                    n = lib.axon_stop_nrt_profile(str(output_dir).encode())
                    print(f"ntff profile: {n} file(s) -> {output_dir}", file=sys.stderr)

            hook = _hook

    mod = types.ModuleType("antenv.axon_hooks")
    mod.get_axon_ntff_profile_hook = lambda: hook
    mod.set_axon_ntff_profile_hook = lambda h: None
    import antenv

    antenv.axon_hooks = mod
    sys.modules["antenv.axon_hooks"] = mod


# ---------------------------------------------------------------------------
# Semaphore count schedule (precomputed; emission asserts it matches).
#
# DVE incs (every DVE instr incs sem_dve):
#   v0=1, hs0=2; per t in 1..15: u,v,Dp,hs = base+1..base+4 with
#   base = 2 + 4*(t-1) + (1 if t >= 10 else 0)   [bc1 sits after hs(9)]
# ACT incs: tq0=1, th0=2; per t: tqf=4t-1, tqgi=4t, tqo=4t+1, th=4t+2
# PE incs (f/gi/o stops + head chunks); PE group order is f, g, i, o: the
# g/i results feed the deep D-chain so they must land early, and the o-group
# matmuls overlap the chain (o's tanh is only needed at the very end):
#   t<=9:  f=3t-2, gi=3t-1, o=3t ; head1=28
#   t>=10: f=3t-1, gi=3t,   o=3t+1 ; head2=47
#
# NOTE: the DVE does NOT interlock same-engine read-after-write — an
# instruction reading the previous instruction's output must carry an
# explicit wait on the DVE's own semaphore (D' below).
# ---------------------------------------------------------------------------
def _dve_base(t):
    return 2 + 4 * (t - 1) + (1 if t >= 10 else 0)


DVE_V0, DVE_HS0 = 1, 2
DVE_U = {t: _dve_base(t) + 1 for t in range(1, T)}
DVE_V = {t: _dve_base(t) + 2 for t in range(1, T)}
DVE_DP = {t: _dve_base(t) + 3 for t in range(1, T)}
DVE_HS = {t: _dve_base(t) + 4 for t in range(1, T)}
DVE_BC1 = DVE_HS[9] + 1  # 39
DVE_BC2 = DVE_HS[15] + 1  # 64

ACT_TQ0, ACT_TH0 = 1, 2
ACT_TQF = {t: 4 * t - 1 for t in range(1, T)}
ACT_TQGI = {t: 4 * t for t in range(1, T)}
ACT_TQO = {t: 4 * t + 1 for t in range(1, T)}
ACT_TH = {t: 4 * t + 2 for t in range(1, T)}

PE_F = {t: (3 * t - 2 if t <= 9 else 3 * t - 1) for t in range(1, T)}
PE_GI = {t: (3 * t - 1 if t <= 9 else 3 * t) for t in range(1, T)}
PE_O = {t: (3 * t if t <= 9 else 3 * t + 1) for t in range(1, T)}
PE_HEAD1 = 28
PE_HEAD2 = 47


def _build():
    from contextlib import ExitStack

    import concourse.bacc as bacc
    import concourse.bass as bass
    import concourse.mybir as mybir

    f32 = mybir.dt.float32
    f16 = mybir.dt.float16
    wdt = mybir.dt.float8e3 if WDT == "f8" else f16
    AF = mybir.ActivationFunctionType
    ADD = mybir.AluOpType.add
    MUL = mybir.AluOpType.mult

    nc = bacc.Bacc("TRN2", target_bir_lowering=False, debug=False, enable_asserts=False)

    wT_d = nc.dram_tensor("wT", [128, 8192], wdt, kind="ExternalInput")
    woT_d = nc.dram_tensor("woT", [128, 16], f16, kind="ExternalInput")
    aux_d = nc.dram_tensor("aux", [8, 532], f16, kind="ExternalInput")
    cst_d = nc.dram_tensor("cst", [128, 16], f32, kind="ExternalInput")
    out_d = nc.dram_tensor("out", [16, 512], f32, kind="ExternalOutput")

    es = ExitStack()
    wT = es.enter_context(nc.sbuf_tensor("wTs", [128, 8192], wdt))
    woT = es.enter_context(nc.sbuf_tensor("woTs", [128, 16], f16))
    aux = es.enter_context(nc.sbuf_tensor("auxs", [8, 532], f16))
    cst = es.enter_context(nc.sbuf_tensor("csts", [128, 16], f32))
    tqt = es.enter_context(nc.sbuf_tensor("tqt", [128, 16], f32))
    ut = es.enter_context(nc.sbuf_tensor("ut", [128, 4], f32))
    vt = es.enter_context(nc.sbuf_tensor("vt", [128, 4], f32))
    Dt = es.enter_context(nc.sbuf_tensor("Dt", [128, 4], f32))
    hs = es.enter_context(nc.sbuf_tensor("hss", [128, 4 * T], f16))
    bco1 = es.enter_context(nc.sbuf_tensor("bco1", [8, 512], f32))
    bco2 = es.enter_context(nc.sbuf_tensor("bco2", [8, 512], f32))
    warm = es.enter_context(nc.sbuf_tensor("warms", [1, 2], f32))
    dumw = es.enter_context(nc.sbuf_tensor("dumw", [128, 128], f16))

    # GATES spans banks 0-2: f in bank 0 cols 0:4, g+i in bank 1 cols 0:8
    # (GATES cols 512:520), o in bank 2 cols 0:4 (GATES cols 1024:1028).
    # ACT reads a bank only after all its matmuls stopped; PE is then writing
    # a different bank, so no PSUM R/W collisions.
    GATES = nc.place_psum_tensor("GATES", [128, 1536], f32, bank=0)
    H1 = nc.place_psum_tensor("H1", [8, 4], f32, bank=3)
    H2 = nc.place_psum_tensor("H2", [8, 4], f32, bank=4)
    # tanh(cx) output lives in PSUM: ScE's PSUM write port is lower-latency
    # than its SBUF port, and nothing else touches bank 5
    tht = nc.place_psum_tensor("THB", [128, 4], f32, bank=5)
    DUM = nc.place_psum_tensor("DUM", [128, 1], f32, bank=6)

    # one semaphore per input DMA: completions of different DMAs interleave
    # across the 16 SDMA engines, so a shared counter cannot order them
    sem_dw = [es.enter_context(nc.semaphore(f"sem_dw{k}")) for k in range(6)]
    sem_aux = es.enter_context(nc.semaphore("sem_aux"))
    sem_cst = es.enter_context(nc.semaphore("sem_cst"))
    sem_do = es.enter_context(nc.semaphore("sem_do"))
    sem_pe = es.enter_context(nc.semaphore("sem_pe"))
    sem_act = es.enter_context(nc.semaphore("sem_act"))
    sem_dve = es.enter_context(nc.semaphore("sem_dve"))

    # PSUM column of gate group g (0=f, 1=g, 2=i, 3=o), tile column c
    GCOL0 = {0: 0, 1: 512, 2: 516, 3: 1024}

    def gates_cols(g, c0, n):
        return GATES[:, GCOL0[g] + c0 : GCOL0[g] + c0 + n]

    inv_s = 1.0 / WSCALE

    with nc.Block() as block:

        @block.tensor
        def _(tensor):
            pe_cnt = 0
            # --- HAM warm-up: dummy pairs on the dummy bank while DMA runs
            for _i in range(N_WARM_MM):
                tensor.matmul(
                    DUM[:, 0:1],
                    dumw[:, 0:128],
                    dumw[:, 0:1],
                    start=True,
                    stop=True,
                    skip_group_check=True,
                )

            def bias_f():
                tensor.matmul(
                    GATES[:, 0:4], aux[0:4, 20:148], aux[0:4, 0:4],
                    start=True, stop=False, skip_group_check=True,
                )

            def bias_gi():
                tensor.matmul(
                    GATES[:, 512:520], aux[0:8, 148:276], aux[0:8, 0:8],
                    start=True, stop=False, skip_group_check=True,
                )

            def bias_o():
                tensor.matmul(
                    GATES[:, 1024:1028], aux[0:4, 276:404], aux[0:4, 0:4],
                    start=True, stop=False, skip_group_check=True,
                )

            def group_mms(t, g, inc=True, stop=True):
                nonlocal pe_cnt
                for c in range(4):
                    for ko in range(4):
                        tile = g * 16 + c * 4 + ko
                        mm = tensor.matmul(
                            gates_cols(g, c, 1),
                            wT[:, tile * 128 : tile * 128 + 128],
                            hs[:, 4 * (t - 1) + ko : 4 * (t - 1) + ko + 1],
                            start=False,
                            stop=(stop and c == 3 and ko == 3),
                            skip_group_check=True,
                        )
                if inc:
                    mm.then_inc(sem_pe)
                    pe_cnt += 1
                return pe_cnt

            def head_mms(trange, HB, mark):
                nonlocal pe_cnt
                t0 = trange[0]
                for ko in range(4):
                    tensor.matmul(
                        HB[0:8, 0:4],
                        hs[:, 4 * t0 + ko : 4 * t0 + ko + 29 : 4],  # [128, 8]
                        woT[:, 4 * ko : 4 * ko + 4],
                        start=(ko == 0),
                        stop=False,
                        skip_group_check=True,
                    )
                mmb = tensor.matmul(
                    HB[0:8, 0:4],
                    aux[0:1, 8:16],
                    aux[0:1, 16:20],
                    start=False,
                    stop=True,
                    skip_group_check=True,
                )
                mmb.then_inc(sem_pe)
                pe_cnt += 1
                assert pe_cnt == mark, (pe_cnt, mark)

            for t in range(1, T):
                if t == 1:
                    # step 1 interleaves bias matmuls with the DMA pacing
                    tensor.wait_ge(sem_aux, 16)  # identities + biasT
                    bias_f()
                    tensor.wait_ge(sem_dw[0], 16)
                    tensor.wait_ge(sem_dve, DVE_HS0)
                    c = group_mms(t, 0)
                    bias_gi()
                    tensor.wait_ge(sem_dw[1], 16)
                    group_mms(t, 1, inc=False, stop=False)
                    tensor.wait_ge(sem_dw[2], 16)
                    group_mms(t, 2)
                    bias_o()
                    tensor.wait_ge(sem_dw[3], 16)
                    c = group_mms(t, 3)
                else:
                    # biases were pre-issued during step t-1's tail
                    tensor.wait_ge(sem_dve, DVE_HS[t - 1])
                    c = group_mms(t, 0)
                    assert c == PE_F[t], (t, c, PE_F[t])
                    group_mms(t, 1, inc=False, stop=False)
                    c = group_mms(t, 2)
                    assert c == PE_GI[t], (t, c, PE_GI[t])
                    c = group_mms(t, 3)
                assert c == PE_O[t], (t, c, PE_O[t])
                if t == 9:
                    tensor.wait_ge(sem_dw[4], 16)  # woT
                    head_mms(range(0, 8), H1, PE_HEAD1)
                if t < T - 1:
                    # pre-issue step t+1's bias matmuls into the PE-idle tail.
                    # Each bank may only be rewritten after step t's ACT read
                    # of that bank completed.
                    tensor.wait_ge(sem_act, ACT_TQF[t])
                    bias_f()
                    tensor.wait_ge(sem_act, ACT_TQGI[t])
                    bias_gi()
                    tensor.wait_ge(sem_act, ACT_TQO[t])
                    bias_o()
            tensor.wait_ge(sem_dve, DVE_HS[15])
            head_mms(range(8, 16), H2, PE_HEAD2)

        @block.scalar
        def _(scalar):
            act_cnt = 0

            def act(out, in_, scale, wait=None):
                nonlocal act_cnt
                if wait is not None:
                    scalar.wait_ge(*wait)
                a = scalar.activation(out, in_, AF.Tanh, scale=scale)
                a.then_inc(sem_act)
                act_cnt += 1
                return act_cnt

            # cst + aux ride the ACT HWDGE ring, parallel to weights on SP
            scalar.dma_start(cst[:], cst_d[:]).then_inc(sem_cst, 16)
            scalar.dma_start(aux[:], aux_d[:]).then_inc(sem_aux, 16)
            # warm the tanh table during the DMA window (input uninitialized;
            # only the table load matters, the output is never read)
            scalar.activation(warm[0:1, 1:2], warm[0:1, 0:1], AF.Tanh)
            # step 0: gates are the constant cst (true units)
            c = act(tqt[:, 0:16], cst[:, 0:16], 1.0, wait=(sem_cst, 16))
            assert c == ACT_TQ0
            c = act(tht[:, 0:4], Dt[:, 0:4], 0.5, wait=(sem_dve, DVE_V0))
            assert c == ACT_TH0
            for t in range(1, T):
                c = act(tqt[:, 0:4], GATES[:, 0:4], inv_s, wait=(sem_pe, PE_F[t]))
                assert c == ACT_TQF[t]
                c = act(tqt[:, 4:12], GATES[:, 512:520], inv_s,
                        wait=(sem_pe, PE_GI[t]))
                assert c == ACT_TQGI[t]
                c = act(tqt[:, 12:16], GATES[:, 1024:1028], inv_s,
                        wait=(sem_pe, PE_O[t]))
                assert c == ACT_TQO[t]
                c = act(tht[:, 0:4], Dt[:, 0:4], 0.5, wait=(sem_dve, DVE_DP[t]))
                assert c == ACT_TH[t]

        @block.vector
        def _(vector):
            dve_cnt = 0

            def stt(out, in0, scalar_imm, in1, op0, op1, wait=None):
                nonlocal dve_cnt
                if wait is not None:
                    vector.wait_ge(*wait)
                i = vector.scalar_tensor_tensor(out, in0, scalar_imm, in1, op0, op1)
                i.then_inc(sem_dve)
                dve_cnt += 1
                return dve_cnt

            tq_f = tqt[:, 0:4]
            tq_g = tqt[:, 4:8]
            tq_i = tqt[:, 8:12]
            tq_o = tqt[:, 12:16]

            # step 0: D0 = (1+ti)*tg ; h2_0 = (1+to)*th0
            c = stt(Dt[:, 0:4], tq_i, 1.0, tq_g, ADD, MUL, wait=(sem_act, ACT_TQ0))
            assert c == DVE_V0
            c = stt(hs[:, 0:4], tq_o, 1.0, tht[:, 0:4], ADD, MUL, wait=(sem_act, ACT_TH0))
            assert c == DVE_HS0

            def bc(HB, dst, mark_pe, mark_dve):
                nonlocal dve_cnt
                vector.wait_ge(sem_pe, mark_pe)
                base = HB[0:8, 0:4]
                rep = bass.AP(base.tensor, base.offset, [list(base.ap[0]), [0, 128], [1, 4]])
                i = vector.tensor_copy(dst.rearrange("p (b d) -> p b d", d=4), rep)
                i.then_inc(sem_dve)
                dve_cnt += 1
                assert dve_cnt == mark_dve, (dve_cnt, mark_dve)

            for t in range(1, T):
                c = stt(ut[:, 0:4], tq_f, 1.0, Dt[:, 0:4], ADD, MUL,
                        wait=(sem_act, ACT_TQF[t]))
                assert c == DVE_U[t]
                c = stt(vt[:, 0:4], tq_i, 1.0, tq_g, ADD, MUL,
                        wait=(sem_act, ACT_TQGI[t]))
                assert c == DVE_V[t]
                # D' = 0.5*u + v; reads the two preceding DVE outputs, so it
                # must wait for v's completion on the DVE's own semaphore
                c = stt(Dt[:, 0:4], ut[:, 0:4], 0.5, vt[:, 0:4], MUL, ADD,
                        wait=(sem_dve, DVE_V[t]))
                assert c == DVE_DP[t]
                c = stt(hs[:, 4 * t : 4 * t + 4], tq_o, 1.0, tht[:, 0:4], ADD, MUL,
                        wait=(sem_act, ACT_TH[t]))
                assert c == DVE_HS[t]
                if t == 9:
                    bc(H1, bco1[:], PE_HEAD1, DVE_BC1)
            bc(H2, bco2[:], PE_HEAD2, DVE_BC2)

        @block.sync
        def _(sync):
            # weight chunks head the SP ring; cst+aux ride the ACT ring in
            # parallel so c1 starts as early as possible
            for k in range(4):
                sync.dma_start(
                    wT[:, k * 2048 : (k + 1) * 2048],
                    wT_d[:, k * 2048 : (k + 1) * 2048],
                ).then_inc(sem_dw[k], 16)
            sync.dma_start(woT[:], woT_d[:]).then_inc(sem_dw[4], 16)
            sync.wait_ge(sem_dve, DVE_BC1)
            sync.dma_start(out_d[0:8, :], bco1[:]).then_inc(sem_do, 16)
            sync.wait_ge(sem_dve, DVE_BC2)
            sync.dma_start(out_d[8:16, :], bco2[:]).then_inc(sem_do, 16)
            sync.wait_ge(sem_do, 32)

    es.close()
    nc.compile()
    return nc


def prep_inputs(Whh, bih, bhh, Wo, bo):
    """Host-side weight relayout + tanh-reparameterization (all tiny)."""
    Whh = np.asarray(Whh, np.float64)
    b = np.asarray(bih, np.float64) + np.asarray(bhh, np.float64)
    Wo = np.asarray(Wo, np.float64)
    bo = np.asarray(bo, np.float64)
    H = HID

    # torch gate order i,f,g,o -> our group order f,g,i,o
    perm = np.concatenate(
        [np.arange(H, 2 * H), np.arange(2 * H, 3 * H),
         np.arange(0, H), np.arange(3 * H, 4 * H)]
    )
    Wp = Whh[perm]
    bp = b[perm]
    # rows: f,i,o scaled by 0.5 (tanh(x/2) = 2*sigmoid(x)-1); g unscaled
    rs = np.ones(4 * H)
    rs[0:H] = 0.5      # f
    rs[2 * H : 3 * H] = 0.5  # i
    rs[3 * H :] = 0.5  # o
    # columns: h2 = 2h -> fold 0.5 into columns
    W2 = rs[:, None] * Wp * 0.5
    b2 = rs * bp

    # gate weight tiles: tile (g,c,ko) at cols (g*16+c*4+ko)*128,
    # lhsT[k, m] = W2[512g+128c+m, 128ko+k]
    Wr = W2.reshape(4, 4, 128, 4, 128)  # [g, c, m, ko, k]
    wTm = np.ascontiguousarray(Wr.transpose(4, 0, 1, 3, 2).reshape(128, 8192))
    wTm = wTm * WSCALE
    if WDT == "f8":
        import ml_dtypes

        wT = wTm.astype(ml_dtypes.float8_e3m4)
    else:
        wT = wTm.astype(np.float16)

    # head: out_t = Wo' @ h2 + bo with Wo' = 0.5*Wo
    Wo2 = 0.5 * Wo  # (4, 512)
    woT = np.ascontiguousarray(
        Wo2.reshape(4, 4, 128).transpose(2, 1, 0).reshape(128, 16)
    ).astype(np.float16)

    # aux [8, 532] f16:
    #   [0:8, 0:8]     I8 (top-left 4x4 doubles as I4)
    #   [0,   8:16]    ones8 (head bias lhsT)
    #   [0,   16:20]   bo
    #   [0:4, 20:148]  biasT for f bank    (scaled by WSCALE)
    #   [0:8, 148:276] biasT for g+i bank
    #   [0:4, 276:404] biasT for o bank
    auxm = np.zeros((8, 532), np.float64)
    auxm[0:8, 0:8] = np.eye(8)
    auxm[0, 8:16] = 1.0
    auxm[0, 16:20] = bo
    bs = (WSCALE * b2).reshape(4, 4, 128)  # [group(f,g,i,o), col, dim]
    auxm[0:4, 20:148] = bs[0]
    auxm[0:4, 148:276] = bs[1]
    auxm[4:8, 148:276] = bs[2]
    auxm[0:4, 276:404] = bs[3]
    aux = auxm.astype(np.float16)

    # cst: step-0 ACT input in true units: [128, 16], col j = dims 128j..128j+127
    cstm = np.ascontiguousarray(b2.reshape(16, 128).T).astype(np.float32)
    return {"wT": wT, "woT": woT, "aux": aux, "cst": cstm}


def kernel(**inputs) -> np.ndarray:
    global last_results
    from concourse.bass_utils import run_bass_kernel_spmd

    if "nc" not in _BUILT:
        _BUILT["nc"] = _build()
    nc = _BUILT["nc"]

    in_map = prep_inputs(
        inputs["Whh"], inputs["bih"], inputs["bhh"], inputs["Wo"], inputs["bo"]
    )
    if os.environ.get("BASS_TRACE"):
        _ensure_ntff_hook()
    in_maps = [dict(in_map) for _ in range(N_CORES)]
    res = run_bass_kernel_spmd(
        nc,
        in_maps,
        core_ids=list(range(N_CORES)),
        trace=bool(os.environ.get("BASS_TRACE")),
    )
    last_results = res
    # out rows: [16, 512] -> (16, 128, 4); concat batch shards
    return np.concatenate(
        [r["out"].reshape(T, BSH, 4) for r in res.results], axis=1
    )


# revision 60
# speedup vs baseline: 1.0127x; 1.0127x over previous
"""Trainium2 Bass kernel for nn_ESBN_77352361001553 (scatter_memory).

Math: the conv encoder is dead code and the LSTM input is constant zeros, so
every batch row follows the identical 16-step, 512-dim LSTM trajectory from
zero state. Output (16, 1024, 4) = broadcast of out_t = Wo @ h_t + bo across
the batch; each of the 8 cores produces the same (16, 512)-f32 row block and
the host reshapes/concats to (16, 1024, 4).

Hand-scheduled raw-bass kernel (no Tile framework), ~58us vs the 73us Tile
baseline. Per-step period ~2.68us = f+g+i matmul phase (48 pairs @27ns,
the N=1 matmul issue floor) + the serial tail (PSUM drain+sem 0.27,
tanh_gi 0.30, v 0.19, D' 0.20, tanh(D/2) 0.27, h-update 0.19, sems ~0.15);
the o-group matmuls, tanh_o, and the NEXT step's three bias matmuls
(pre-issued, gated on the current step's ACT reads of their banks) all
overlap the tail, so each step opens directly with its f-matmuls.
Design notes:
 - tanh-only gates: host pre-scales f/i/o rows of Whh by 0.5 so
   tanh(pre) = 2*sigmoid(gate)-1, and tracks D = 2*cx, h2 = 2*h (the factor
   is folded into Whh's columns and Wo). Per step ACT runs 4 instrs
   (tanh_f, tanh_{g,i}, tanh_o, tanh(D/2)); DVE runs 4 fused
   scalar_tensor_tensor ops:
     u = (1+tf)*D ; v = (1+ti)*tg ; D' = 0.5*u + v ; h2 = (1+to)*th
 - weights are float8 e3m4 scaled by 16 (host-quantized; rhs stays fp16 —
   mixed-dtype matmul works); the 1/16 rides the tanh ACT scale for free.
   This halves the weight-DMA ramp; rel err 6.1e-3 (tolerance 2e-2).
 - gate biases enter PSUM via K=4/K=8 (bias^T, I) matmuls opening each
   bank's accumulation group (no bias-add on the critical path).
 - three PSUM gate banks (f | g+i | o): ACT reads a bank only after its
   matmuls stop, while PE writes a different bank (PSUM R/W collision is
   fatal); tanh(D/2) lands in its own PSUM bank (ScE->PSUM is faster).
 - head computed in two [8,4] PSUM chunks (steps 0-7 during step 9, steps
   8-15 at the end) so most of the output DMA overlaps compute.
 - hand-placed semaphores, one per producer stream + one per input DMA
   (completions of different DMAs interleave across the 16 SDMA engines, so
   a shared counter cannot order them). Every instruction carries <=1 wait.
 - the DVE does NOT interlock same-engine read-after-write: any DVE op
   reading the previous DVE op's output needs a wait on the DVE's own
   semaphore (D' above). Engine-FIFO completion order is still in-order.
 - standalone engine.wait_ge() fuses into the next instruction; on the PE
   that is the LDWEIGHTS half of a matmul, which is exactly where DMA
   gating must sit (the PE reorder window pulls LDWEIGHTS ahead of waiting
   matmuls).
Remaining fixed costs outside our control: ~1.4us of framework preamble
inside the measured window, and ~7.3us of neuronxcc-emitted full
semaphore-file resets + barriers at kernel end (the Tile baseline pays the
same).
"""

import os

import numpy as np

T = 16
HID = 512
N_CORES = 8
BSH = 128  # batch shard per core

WDT = os.environ.get("KERNEL_WDT", "f8")  # "f16" or "f8" (float8e3 = e3m4)
WSCALE = 16.0 if WDT == "f8" else 1.0
N_WARM_MM = int(os.environ.get("KERNEL_WARM", "56"))

_BUILT = {}
last_results = None  # BassKernelResults of the most recent run (for tooling)


def _ensure_ntff_hook():
    """Register the axon NTFF profiling hook if the container lacks
    antenv.axon_hooks (slim boot)."""
    import contextlib
    import ctypes
    import sys
    import types

    try:
        from antenv.axon_hooks import get_axon_ntff_profile_hook  # noqa: F401

        return
    except ImportError:
        pass

    so_path = "/opt/axon/libaxon_pjrt.so"
    hook = None
    if os.path.exists(so_path):
        lib = ctypes.CDLL(so_path)
        if hasattr(lib, "axon_start_nrt_profile"):
            lib.axon_start_nrt_profile.argtypes = [
                ctypes.POINTER(ctypes.c_int64),
                ctypes.c_size_t,
            ]
            lib.axon_start_nrt_profile.restype = ctypes.c_int64
            lib.axon_stop_nrt_profile.argtypes = [ctypes.c_char_p]
            lib.axon_stop_nrt_profile.restype = ctypes.c_int64

            @contextlib.contextmanager
            def _hook(output_dir, device_ids):
                import jax

                jax.devices()
                if device_ids:
                    ids = (ctypes.c_int64 * len(device_ids))(*device_ids)
                    rc = lib.axon_start_nrt_profile(ids, len(device_ids))
                else:
                    rc = lib.axon_start_nrt_profile(None, 0)
                if rc != 0:
                    raise RuntimeError(f"axon_start_nrt_profile rc={rc}")
                try:
                    yield
                finally:
                    n = lib.axon_stop_nrt_profile(str(output_dir).encode())
                    print(f"ntff profile: {n} file(s) -> {output_dir}", file=sys.stderr)

            hook = _hook

    mod = types.ModuleType("antenv.axon_hooks")
    mod.get_axon_ntff_profile_hook = lambda: hook
    mod.set_axon_ntff_profile_hook = lambda h: None
    import antenv

    antenv.axon_hooks = mod
    sys.modules["antenv.axon_hooks"] = mod


# ---------------------------------------------------------------------------
# Semaphore count schedule (precomputed; emission asserts it matches).
#
# DVE incs (every DVE instr incs sem_dve):
#   v0=1, hs0=2; per t in 1..15: u,v,Dp,hs = base+1..base+4 with
#   base = 2 + 4*(t-1) + (1 if t >= 10 else 0)   [bc1 sits after hs(9)]
# ACT incs: tq0=1, th0=2; per t: tqf=4t-1, tqgi=4t, tqo=4t+1, th=4t+2
# PE incs (f/gi/o stops + head chunks); PE group order is f, g, i, o: the
# g/i results feed the deep D-chain so they must land early, and the o-group
# matmuls overlap the chain (o's tanh is only needed at the very end):
#   t<=9:  f=3t-2, gi=3t-1, o=3t ; head1=28
#   t>=10: f=3t-1, gi=3t,   o=3t+1 ; head2=47
#
# NOTE: the DVE does NOT interlock same-engine read-after-write — an
# instruction reading the previous instruction's output must carry an
# explicit wait on the DVE's own semaphore (D' below).
# ---------------------------------------------------------------------------
def _dve_base(t):
    return 2 + 4 * (t - 1) + (1 if t >= 10 else 0)


DVE_V0, DVE_HS0 = 1, 2
DVE_U = {t: _dve_base(t) + 1 for t in range(1, T)}
DVE_V = {t: _dve_base(t) + 2 for t in range(1, T)}
DVE_DP = {t: _dve_base(t) + 3 for t in range(1, T)}
DVE_HS = {t: _dve_base(t) + 4 for t in range(1, T)}
DVE_BC1 = DVE_HS[9] + 1  # 39
DVE_BC2 = DVE_HS[15] + 1  # 64

ACT_TQ0, ACT_TH0 = 1, 2
ACT_TQF = {t: 4 * t - 1 for t in range(1, T)}
ACT_TQGI = {t: 4 * t for t in range(1, T)}
ACT_TQO = {t: 4 * t + 1 for t in range(1, T)}
ACT_TH = {t: 4 * t + 2 for t in range(1, T)}

PE_F = {t: (3 * t - 2 if t <= 9 else 3 * t - 1) for t in range(1, T)}
PE_GI = {t: (3 * t - 1 if t <= 9 else 3 * t) for t in range(1, T)}
PE_O = {t: (3 * t if t <= 9 else 3 * t + 1) for t in range(1, T)}
PE_HEAD1 = 28
PE_HEAD2 = 47


def _build():
    from contextlib import ExitStack

    import concourse.bacc as bacc
    import concourse.bass as bass
    import concourse.mybir as mybir

    f32 = mybir.dt.float32
    f16 = mybir.dt.float16
    wdt = mybir.dt.float8e3 if WDT == "f8" else f16
    AF = mybir.ActivationFunctionType
    ADD = mybir.AluOpType.add
    MUL = mybir.AluOpType.mult

    nc = bacc.Bacc("TRN2", target_bir_lowering=False, debug=False, enable_asserts=False)

    wT_d = nc.dram_tensor("wT", [128, 8192], wdt, kind="ExternalInput")
    woT_d = nc.dram_tensor("woT", [128, 16], f16, kind="ExternalInput")
    aux_d = nc.dram_tensor("aux", [8, 532], f16, kind="ExternalInput")
    cst_d = nc.dram_tensor("cst", [128, 16], f32, kind="ExternalInput")
    out_d = nc.dram_tensor("out", [16, 512], f32, kind="ExternalOutput")

    es = ExitStack()
    wT = es.enter_context(nc.sbuf_tensor("wTs", [128, 8192], wdt))
    woT = es.enter_context(nc.sbuf_tensor("woTs", [128, 16], f16))
    aux = es.enter_context(nc.sbuf_tensor("auxs", [8, 532], f16))
    cst = es.enter_context(nc.sbuf_tensor("csts", [128, 16], f32))
    tqt = es.enter_context(nc.sbuf_tensor("tqt", [128, 16], f32))
    ut = es.enter_context(nc.sbuf_tensor("ut", [128, 4], f32))
    vt = es.enter_context(nc.sbuf_tensor("vt", [128, 4], f32))
    hs = es.enter_context(nc.sbuf_tensor("hss", [128, 4 * T], f16))
    bco1 = es.enter_context(nc.sbuf_tensor("bco1", [8, 512], f32))
    bco2 = es.enter_context(nc.sbuf_tensor("bco2", [8, 512], f32))
    warm = es.enter_context(nc.sbuf_tensor("warms", [1, 2], f32))
    dumw = es.enter_context(nc.sbuf_tensor("dumw", [128, 128], f16))

    # GATES spans banks 0-2: f in bank 0 cols 0:4, g+i in bank 1 cols 0:8
    # (GATES cols 512:520), o in bank 2 cols 0:4 (GATES cols 1024:1028).
    # ACT reads a bank only after all its matmuls stopped; PE is then writing
    # a different bank, so no PSUM R/W collisions.
    GATES = nc.place_psum_tensor("GATES", [128, 1536], f32, bank=0)
    H1 = nc.place_psum_tensor("H1", [8, 4], f32, bank=3)
    H2 = nc.place_psum_tensor("H2", [8, 4], f32, bank=4)
    # tanh(cx) output lives in PSUM: ScE's PSUM write port is lower-latency
    # than its SBUF port, and nothing else touches bank 5. The doubled cell
    # state D sits in bank 7 so tanh(D/2) reads AND writes on ScE's fast side.
    tht = nc.place_psum_tensor("THB", [128, 4], f32, bank=5)
    DUM = nc.place_psum_tensor("DUM", [128, 1], f32, bank=6)
    Dt = nc.place_psum_tensor("DTB", [128, 4], f32, bank=7)

    # one semaphore per input DMA: completions of different DMAs interleave
    # across the 16 SDMA engines, so a shared counter cannot order them
    sem_dw = [es.enter_context(nc.semaphore(f"sem_dw{k}")) for k in range(6)]
    sem_aux = es.enter_context(nc.semaphore("sem_aux"))
    sem_cst = es.enter_context(nc.semaphore("sem_cst"))
    sem_do = es.enter_context(nc.semaphore("sem_do"))
    sem_pe = es.enter_context(nc.semaphore("sem_pe"))
    sem_act = es.enter_context(nc.semaphore("sem_act"))
    sem_dve = es.enter_context(nc.semaphore("sem_dve"))

    # PSUM column of gate group g (0=f, 1=g, 2=i, 3=o), tile column c
    GCOL0 = {0: 0, 1: 512, 2: 516, 3: 1024}

    def gates_cols(g, c0, n):
        return GATES[:, GCOL0[g] + c0 : GCOL0[g] + c0 + n]

    inv_s = 1.0 / WSCALE

    with nc.Block() as block:

        @block.tensor
        def _(tensor):
            pe_cnt = 0
            # --- HAM warm-up: dummy pairs on the dummy bank while DMA runs
            for _i in range(N_WARM_MM):
                tensor.matmul(
                    DUM[:, 0:1],
                    dumw[:, 0:128],
                    dumw[:, 0:1],
                    start=True,
                    stop=True,
                    skip_group_check=True,
                )

            def bias_f():
                tensor.matmul(
                    GATES[:, 0:4], aux[0:4, 20:148], aux[0:4, 0:4],
                    start=True, stop=False, skip_group_check=True,
                )

            def bias_gi():
                tensor.matmul(
                    GATES[:, 512:520], aux[0:8, 148:276], aux[0:8, 0:8],
                    start=True, stop=False, skip_group_check=True,
                )

            def bias_o():
                tensor.matmul(
                    GATES[:, 1024:1028], aux[0:4, 276:404], aux[0:4, 0:4],
                    start=True, stop=False, skip_group_check=True,
                )

            def group_mms(t, g, inc=True, stop=True):
                nonlocal pe_cnt
                for c in range(4):
                    for ko in range(4):
                        tile = g * 16 + c * 4 + ko
                        mm = tensor.matmul(
                            gates_cols(g, c, 1),
                            wT[:, tile * 128 : tile * 128 + 128],
                            hs[:, 4 * (t - 1) + ko : 4 * (t - 1) + ko + 1],
                            start=False,
                            stop=(stop and c == 3 and ko == 3),
                            skip_group_check=True,
                        )
                if inc:
                    mm.then_inc(sem_pe)
                    pe_cnt += 1
                return pe_cnt

            def head_mms(trange, HB, mark):
                nonlocal pe_cnt
                t0 = trange[0]
                for ko in range(4):
                    tensor.matmul(
                        HB[0:8, 0:4],
                        hs[:, 4 * t0 + ko : 4 * t0 + ko + 29 : 4],  # [128, 8]
                        woT[:, 4 * ko : 4 * ko + 4],
                        start=(ko == 0),
                        stop=False,
                        skip_group_check=True,
                    )
                mmb = tensor.matmul(
                    HB[0:8, 0:4],
                    aux[0:1, 8:16],
                    aux[0:1, 16:20],
                    start=False,
                    stop=True,
                    skip_group_check=True,
                )
                mmb.then_inc(sem_pe)
                pe_cnt += 1
                assert pe_cnt == mark, (pe_cnt, mark)

            for t in range(1, T):
                if t == 1:
                    # step 1 interleaves bias matmuls with the DMA pacing
                    tensor.wait_ge(sem_aux, 16)  # identities + biasT
                    bias_f()
                    tensor.wait_ge(sem_dw[0], 16)
                    tensor.wait_ge(sem_dve, DVE_HS0)
                    c = group_mms(t, 0)
                    bias_gi()
                    tensor.wait_ge(sem_dw[1], 16)
                    group_mms(t, 1, inc=False, stop=False)
                    tensor.wait_ge(sem_dw[2], 16)
                    group_mms(t, 2)
                    bias_o()
                    tensor.wait_ge(sem_dw[3], 16)
                    c = group_mms(t, 3)
                else:
                    # biases were pre-issued during step t-1's tail
                    tensor.wait_ge(sem_dve, DVE_HS[t - 1])
                    c = group_mms(t, 0)
                    assert c == PE_F[t], (t, c, PE_F[t])
                    group_mms(t, 1, inc=False, stop=False)
                    c = group_mms(t, 2)
                    assert c == PE_GI[t], (t, c, PE_GI[t])
                    c = group_mms(t, 3)
                assert c == PE_O[t], (t, c, PE_O[t])
                if t == 9:
                    tensor.wait_ge(sem_dw[4], 16)  # woT
                    head_mms(range(0, 8), H1, PE_HEAD1)
                if t < T - 1:
                    # pre-issue step t+1's bias matmuls into the PE-idle tail.
                    # Each bank may only be rewritten after step t's ACT read
                    # of that bank completed.
                    tensor.wait_ge(sem_act, ACT_TQF[t])
                    bias_f()
                    tensor.wait_ge(sem_act, ACT_TQGI[t])
                    bias_gi()
                    tensor.wait_ge(sem_act, ACT_TQO[t])
                    bias_o()
            tensor.wait_ge(sem_dve, DVE_HS[15])
            head_mms(range(8, 16), H2, PE_HEAD2)

        @block.scalar
        def _(scalar):
            act_cnt = 0

            def act(out, in_, scale, wait=None):
                nonlocal act_cnt
                if wait is not None:
                    scalar.wait_ge(*wait)
                a = scalar.activation(out, in_, AF.Tanh, scale=scale)
                a.then_inc(sem_act)
                act_cnt += 1
                return act_cnt

            # cst + aux ride the ACT HWDGE ring, parallel to weights on SP
            scalar.dma_start(cst[:], cst_d[:]).then_inc(sem_cst, 16)
            scalar.dma_start(aux[:], aux_d[:]).then_inc(sem_aux, 16)
            # warm the tanh table during the DMA window (input uninitialized;
            # only the table load matters, the output is never read)
            scalar.activation(warm[0:1, 1:2], warm[0:1, 0:1], AF.Tanh)
            # step 0: gates are the constant cst (true units)
            c = act(tqt[:, 0:16], cst[:, 0:16], 1.0, wait=(sem_cst, 16))
            assert c == ACT_TQ0
            c = act(tht[:, 0:4], Dt[:, 0:4], 0.5, wait=(sem_dve, DVE_V0))
            assert c == ACT_TH0
            for t in range(1, T):
                c = act(tqt[:, 0:4], GATES[:, 0:4], inv_s, wait=(sem_pe, PE_F[t]))
                assert c == ACT_TQF[t]
                c = act(tqt[:, 4:12], GATES[:, 512:520], inv_s,
                        wait=(sem_pe, PE_GI[t]))
                assert c == ACT_TQGI[t]
                c = act(tqt[:, 12:16], GATES[:, 1024:1028], inv_s,
                        wait=(sem_pe, PE_O[t]))
                assert c == ACT_TQO[t]
                c = act(tht[:, 0:4], Dt[:, 0:4], 0.5, wait=(sem_dve, DVE_DP[t]))
                assert c == ACT_TH[t]

        @block.vector
        def _(vector):
            dve_cnt = 0

            def stt(out, in0, scalar_imm, in1, op0, op1, wait=None):
                nonlocal dve_cnt
                if wait is not None:
                    vector.wait_ge(*wait)
                i = vector.scalar_tensor_tensor(out, in0, scalar_imm, in1, op0, op1)
                i.then_inc(sem_dve)
                dve_cnt += 1
                return dve_cnt

            tq_f = tqt[:, 0:4]
            tq_g = tqt[:, 4:8]
            tq_i = tqt[:, 8:12]
            tq_o = tqt[:, 12:16]

            # step 0: D0 = (1+ti)*tg ; h2_0 = (1+to)*th0
            c = stt(Dt[:, 0:4], tq_i, 1.0, tq_g, ADD, MUL, wait=(sem_act, ACT_TQ0))
            assert c == DVE_V0
            c = stt(hs[:, 0:4], tq_o, 1.0, tht[:, 0:4], ADD, MUL, wait=(sem_act, ACT_TH0))
            assert c == DVE_HS0

            def bc(HB, dst, mark_pe, mark_dve):
                nonlocal dve_cnt
                vector.wait_ge(sem_pe, mark_pe)
                base = HB[0:8, 0:4]
                rep = bass.AP(base.tensor, base.offset, [list(base.ap[0]), [0, 128], [1, 4]])
                i = vector.tensor_copy(dst.rearrange("p (b d) -> p b d", d=4), rep)
                i.then_inc(sem_dve)
                dve_cnt += 1
                assert dve_cnt == mark_dve, (dve_cnt, mark_dve)

            for t in range(1, T):
                c = stt(ut[:, 0:4], tq_f, 1.0, Dt[:, 0:4], ADD, MUL,
                        wait=(sem_act, ACT_TQF[t]))
                assert c == DVE_U[t]
                c = stt(vt[:, 0:4], tq_i, 1.0, tq_g, ADD, MUL,
                        wait=(sem_act, ACT_TQGI[t]))
                assert c == DVE_V[t]
                # D' = 0.5*u + v; reads the two preceding DVE outputs, so it
                # must wait for v's completion on the DVE's own semaphore
                c = stt(Dt[:, 0:4], ut[:, 0:4], 0.5, vt[:, 0:4], MUL, ADD,
                        wait=(sem_dve, DVE_V[t]))
                assert c == DVE_DP[t]
                c = stt(hs[:, 4 * t : 4 * t + 4], tq_o, 1.0, tht[:, 0:4], ADD, MUL,
                        wait=(sem_act, ACT_TH[t]))
                assert c == DVE_HS[t]
                if t == 9:
                    bc(H1, bco1[:], PE_HEAD1, DVE_BC1)
            bc(H2, bco2[:], PE_HEAD2, DVE_BC2)

        @block.sync
        def _(sync):
            # weight chunks head the SP ring; cst+aux ride the ACT ring in
            # parallel so c1 starts as early as possible
            for k in range(4):
                sync.dma_start(
                    wT[:, k * 2048 : (k + 1) * 2048],
                    wT_d[:, k * 2048 : (k + 1) * 2048],
                ).then_inc(sem_dw[k], 16)
            sync.dma_start(woT[:], woT_d[:]).then_inc(sem_dw[4], 16)
            sync.wait_ge(sem_dve, DVE_BC1)
            sync.dma_start(out_d[0:8, :], bco1[:]).then_inc(sem_do, 16)
            sync.wait_ge(sem_dve, DVE_BC2)
            sync.dma_start(out_d[8:16, :], bco2[:]).then_inc(sem_do, 16)
            sync.wait_ge(sem_do, 32)

    es.close()
    nc.compile()
    return nc


def prep_inputs(Whh, bih, bhh, Wo, bo):
    """Host-side weight relayout + tanh-reparameterization (all tiny)."""
    Whh = np.asarray(Whh, np.float64)
    b = np.asarray(bih, np.float64) + np.asarray(bhh, np.float64)
    Wo = np.asarray(Wo, np.float64)
    bo = np.asarray(bo, np.float64)
    H = HID

    # torch gate order i,f,g,o -> our group order f,g,i,o
    perm = np.concatenate(
        [np.arange(H, 2 * H), np.arange(2 * H, 3 * H),
         np.arange(0, H), np.arange(3 * H, 4 * H)]
    )
    Wp = Whh[perm]
    bp = b[perm]
    # rows: f,i,o scaled by 0.5 (tanh(x/2) = 2*sigmoid(x)-1); g unscaled
    rs = np.ones(4 * H)
    rs[0:H] = 0.5      # f
    rs[2 * H : 3 * H] = 0.5  # i
    rs[3 * H :] = 0.5  # o
    # columns: h2 = 2h -> fold 0.5 into columns
    W2 = rs[:, None] * Wp * 0.5
    b2 = rs * bp

    # gate weight tiles: tile (g,c,ko) at cols (g*16+c*4+ko)*128,
    # lhsT[k, m] = W2[512g+128c+m, 128ko+k]
    Wr = W2.reshape(4, 4, 128, 4, 128)  # [g, c, m, ko, k]
    wTm = np.ascontiguousarray(Wr.transpose(4, 0, 1, 3, 2).reshape(128, 8192))
    wTm = wTm * WSCALE
    if WDT == "f8":
        import ml_dtypes

        wT = wTm.astype(ml_dtypes.float8_e3m4)
    else:
        wT = wTm.astype(np.float16)

    # head: out_t = Wo' @ h2 + bo with Wo' = 0.5*Wo
    Wo2 = 0.5 * Wo  # (4, 512)
    woT = np.ascontiguousarray(
        Wo2.reshape(4, 4, 128).transpose(2, 1, 0).reshape(128, 16)
    ).astype(np.float16)

    # aux [8, 532] f16:
    #   [0:8, 0:8]     I8 (top-left 4x4 doubles as I4)
    #   [0,   8:16]    ones8 (head bias lhsT)
    #   [0,   16:20]   bo
    #   [0:4, 20:148]  biasT for f bank    (scaled by WSCALE)
    #   [0:8, 148:276] biasT for g+i bank
    #   [0:4, 276:404] biasT for o bank
    auxm = np.zeros((8, 532), np.float64)
    auxm[0:8, 0:8] = np.eye(8)
    auxm[0, 8:16] = 1.0
    auxm[0, 16:20] = bo
    bs = (WSCALE * b2).reshape(4, 4, 128)  # [group(f,g,i,o), col, dim]
    auxm[0:4, 20:148] = bs[0]
    auxm[0:4, 148:276] = bs[1]
    auxm[4:8, 148:276] = bs[2]
    auxm[0:4, 276:404] = bs[3]
    aux = auxm.astype(np.float16)

    # cst: step-0 ACT input in true units: [128, 16], col j = dims 128j..128j+127
    cstm = np.ascontiguousarray(b2.reshape(16, 128).T).astype(np.float32)
    return {"wT": wT, "woT": woT, "aux": aux, "cst": cstm}


def kernel(**inputs) -> np.ndarray:
    global last_results
    from concourse.bass_utils import run_bass_kernel_spmd

    if "nc" not in _BUILT:
        _BUILT["nc"] = _build()
    nc = _BUILT["nc"]

    in_map = prep_inputs(
        inputs["Whh"], inputs["bih"], inputs["bhh"], inputs["Wo"], inputs["bo"]
    )
    if os.environ.get("BASS_TRACE"):
        _ensure_ntff_hook()
    in_maps = [dict(in_map) for _ in range(N_CORES)]
    res = run_bass_kernel_spmd(
        nc,
        in_maps,
        core_ids=list(range(N_CORES)),
        trace=bool(os.environ.get("BASS_TRACE")),
    )
    last_results = res
    # out rows: [16, 512] -> (16, 128, 4); concat batch shards
    return np.concatenate(
        [r["out"].reshape(T, BSH, 4) for r in res.results], axis=1
    )
